# revision 38
# baseline (speedup 1.0000x reference)
"""Trainium2 Bass kernel for nn_Attention_18399639896530.

Reference computation (b=2, c=256, l=4096, heads=4, dim_head=32):
  qkv   = w_qkv @ x[b]                  (pointwise conv == channel matmul)
  q,k,v -> (b, h, d, l);  q,k L2-normalized over the *sequence* axis l
  sim   = 10 * q^T k    (per b,h: (l, l))
  attn  = softmax(sim, -1);  out = attn @ v^T   -> (b, h, l, d)
  y     = w_out @ out.reshape(b, 128, l) + b_out
          ^^^ row-major reshape of (h, l, d) -- a scrambled view, NOT a
          transpose: view[h*32+r', t] = out[b, h, r'*128 + t//32, t%32]

Sharding: 8 cores == 8 (b, h) pairs; per-core flash-style attention with the
softmax denominator produced by an extra ones-column in the stationary v^T
operand.  Both L2 norms fold into a single per-row scale of k (they both
scale the d-rows).

The scrambled output projection contracts over r' = i//128.  Query columns
are processed u-major (flash position u*32 + c <-> sequence index c*128 + u,
via a strided view of x in the q projection), so each 32-wide accumulator
slice holds one u with c = 0..31; small [33,32] TensorE transposes then land
o_norm/denominator directly on partition c, DVE normalizes into
R[r', u, dd] = o_norm[dd, r'*128+u], and y_h = wo_h^T.T @ R.  (A previous
revision instead DMA'd [128,32] tiles into single partitions of R -- that
partition-collapsing SBUF->SBUF DMA was 404us of a 486us kernel.)

x/w_qkv/w_v ship as bf16 (same 1 cyc/col as f32r, half the DMA; fp8 fails
the 2e-2 gate because the random-sign 256-channel contraction keeps the full
per-term quantization noise).  Projection square-norms accumulate on ACT
under the projection matmuls; ACT's spline table is warmed so the exp train
never reloads.  Host sums the 4 per-head partials per batch and adds b_out.
"""

import os
import sys
import numpy as np

try:
    import concourse  # noqa: F401
except ImportError:  # pragma: no cover
    sys.path.insert(0, "/opt/trn_rl_repo")

import concourse.bass as bass  # noqa: E402
import concourse.tile as tile  # noqa: E402
from concourse import bacc, mybir  # noqa: E402
from concourse import bass_utils  # noqa: E402
from concourse.masks import make_identity  # noqa: E402

B, C, L = 2, 256, 4096
H, D = 4, 32
IC = 1024          # i-chunk (query columns per block of the flash loop)
NIC = L // IC      # 4
NJ = L // 128      # 32 key blocks
F32 = mybir.dt.float32
F32R = mybir.dt.float32r   # single-pass fp32 matmul: 1 cyc/col at N>=256
BF16 = mybir.dt.bfloat16   # projection inputs ship as bf16: same 1 cyc/col
                           # as f32r, half the DMA + SBUF; fp8 was tried and
                           # fails tolerance (random-sign channel sum keeps
                           # the full ~4% per-term quantization noise)

_CACHE = {}
MM_F32 = bool(int(os.environ.get("MM_F32", "0")))


def _mm(ap):
    # hot-matmul operand dtype: f32r (1 cyc/col if real) vs plain f32 (4 cyc)
    return ap.bitcast(F32) if (MM_F32 and ap.dtype == F32R) else ap


def _emit(tc, y_d, x_d, wqk_d, wv_d, wo_d):
    from contextlib import ExitStack

    nc = tc.nc
    with ExitStack() as ctx:
        const = ctx.enter_context(tc.tile_pool(name="const", bufs=1))
        work = ctx.enter_context(tc.tile_pool(name="work", bufs=2))
        epool = ctx.enter_context(tc.tile_pool(name="epool", bufs=3))
        opool = ctx.enter_context(tc.tile_pool(name="opool", bufs=2))
        psA = ctx.enter_context(tc.tile_pool(name="psA", bufs=1, space="PSUM"))
        psS = ctx.enter_context(tc.tile_pool(name="psS", bufs=2, space="PSUM"))
        psT = ctx.enter_context(tc.tile_pool(name="psT", bufs=2, space="PSUM"))

        # ---- load inputs (small weights first so projection starts early)
        wqk_sb = const.tile([128, 2, 2 * D], BF16)
        nc.sync.dma_start(wqk_sb, wqk_d.rearrange("(cc p) o -> p cc o", p=128))
        wv_sb = const.tile([128, 2, D], BF16)
        nc.sync.dma_start(wv_sb, wv_d.rearrange("(cc p) o -> p cc o", p=128))
        wo_sb = const.tile([D, C], F32R)              # [r', o]
        nc.sync.dma_start(wo_sb, wo_d)
        x_sb = const.tile([128, 2, L], BF16)            # [c%128, c//128, l]
        xr = x_d.rearrange("(cc p) l -> p cc l", p=128)
        for lq in range(8):
            nc.sync.dma_start(x_sb[:, :, lq * 512:(lq + 1) * 512],
                              xr[:, :, lq * 512:(lq + 1) * 512])

        ones_f32 = const.tile([128, D], F32)
        nc.vector.memset(ones_f32, 1.0)
        # Ln-only table warm while the x DMA is in flight: narrows ACT's
        # possible table sets to the Ln-bearing ones, so the Square accums
        # and the f-chain Ln never reload; only the f-chain Exp pays one
        # ACT_TABLE_LOAD before the exp train.
        actwarm = work.tile([D, 1], F32, tag="actwarm")
        nc.scalar.activation(actwarm, ones_f32[0:D, 0:1],
                             mybir.ActivationFunctionType.Ln)
        warm_ps = psT.tile([D, D], F32, name="warm", tag="yp")
        for _ in range(40):
            nc.tensor.matmul(warm_ps, ones_f32, ones_f32, start=True, stop=True)
        ident = const.tile([D + 1, D + 1], F32)
        make_identity(nc, ident)

        # v^T blocks with a trailing ones column: [j%128, jb//4, jb%4, d(+1)]
        vt_sb = const.tile([128, NJ // 4, 4, D + 1], F32R)
        nc.vector.tensor_copy(
            vt_sb[:, :, :, D],
            ones_f32.rearrange("p (g l) -> p g l", l=4))

        q_sb = const.tile([D, L], F32R)
        k_sb = const.tile([D, L], F32R)
        # R[r', u, dd] = o_norm[dd, r'*128 + u]
        R_sb = const.tile([D, 128, D], F32R)

        # q columns are permuted u-major: flash position pos = u*32 + c maps
        # to sequence index i = c*128 + u.  A 32-wide slice of the attention
        # accumulator then holds one u with c=0..31, so a [33,32] transpose
        # lands o_norm directly on partition c = i//128 -- the layout the
        # output projection contracts over -- with no partition-collapsing
        # DMA.  x viewed with free dims (u, c): l = c*128 + u.
        x_perm = x_sb.rearrange("p cc (c u) -> p cc u c", c=D)

        # ---- q/k projection (k first: its chunks consume x in DMA order,
        # overlapping the x load; q's permuted view needs all of x).
        # Square-norm partials accumulate on ACT per chunk, hidden under the
        # projection matmuls instead of serializing before the flash loop.
        nq16 = work.tile([D, L // 512], F32, tag="nq16")
        nk16 = work.tile([D, L // 512], F32, tag="nk16")

        def proj_qk(which, dst, npart):
            for lc in range(L // 512):
                # alternate PSUM pools: 4 pq tiles in flight hides the
                # psum-slot -> copy -> free round-trip
                pool, tag = (psS, "s") if lc % 2 == 0 else (psT, "yp")
                pq = pool.tile([D, 512], F32, tag=tag, name="pq")
                for cc in range(2):
                    rhs = (x_perm[:, cc, lc * 16:(lc + 1) * 16, :]
                           if which == 0 else
                           x_sb[:, cc, lc * 512:(lc + 1) * 512])
                    nc.tensor.matmul(
                        pq, wqk_sb[:, cc, which * D:(which + 1) * D],
                        rhs, start=(cc == 0), stop=(cc == 1))
                sq_scr = work.tile([D, 512], F32, tag="sq")
                nc.scalar.activation(sq_scr, pq,
                                     mybir.ActivationFunctionType.Square,
                                     accum_out=npart[:, lc:lc + 1])
                nc.vector.tensor_copy(dst[:, lc * 512:(lc + 1) * 512], pq)

        # k and v interleaved per 512-col x chunk, so both consume x in DMA
        # order and hide entirely under the load; q last (its u-major
        # permuted view needs all of x).
        def proj_k_chunk(lc):
            pool, tag = (psS, "s") if lc % 2 == 0 else (psT, "yp")
            pq = pool.tile([D, 512], F32, tag=tag, name="pq")
            for cc in range(2):
                nc.tensor.matmul(
                    pq, wqk_sb[:, cc, D:2 * D],
                    x_sb[:, cc, lc * 512:(lc + 1) * 512],
                    start=(cc == 0), stop=(cc == 1))
            sq_scr = work.tile([D, 512], F32, tag="sq")
            nc.scalar.activation(sq_scr, pq,
                                 mybir.ActivationFunctionType.Square,
                                 accum_out=nk16[:, lc:lc + 1])
            nc.vector.tensor_copy(k_sb[:, lc * 512:(lc + 1) * 512], pq)

        def proj_v_group(g):
            vt_ps = psS.tile([128, 4, D], F32, tag="s")
            for l4 in range(4):
                jb = g * 4 + l4
                for cc in range(2):
                    nc.tensor.matmul(
                        vt_ps[:, l4, :],
                        x_sb[:, cc, jb * 128:(jb + 1) * 128], wv_sb[:, cc, :],
                        start=(cc == 0), stop=(cc == 1))
            nc.vector.tensor_copy(vt_sb[:, g, :, 0:D], vt_ps)

        for lc in range(8):
            proj_k_chunk(lc)
            proj_v_group(lc)
        proj_qk(0, q_sb, nq16)

        # ---- fold both L2 norms into one per-row scale of k --------------
        # f[d] = 1 / (max(||q_d||,eps) * max(||k_d||,eps))
        #      = exp(-0.5 * ln(max(sum q_d^2,eps^2) * max(sum k_d^2,eps^2)))
        nqs = work.tile([D, 1], F32)
        nks = work.tile([D, 1], F32)
        nc.vector.tensor_reduce(nqs, nq16, axis=mybir.AxisListType.X,
                                op=mybir.AluOpType.add)
        nc.vector.tensor_reduce(nks, nk16, axis=mybir.AxisListType.X,
                                op=mybir.AluOpType.add)
        nc.vector.tensor_scalar_max(nqs, nqs, 1e-24)
        nc.vector.tensor_scalar_max(nks, nks, 1e-24)
        m = work.tile([D, 1], F32)
        nc.vector.tensor_mul(m, nqs, nks)
        lnm = work.tile([D, 1], F32)
        nc.scalar.activation(lnm, m, mybir.ActivationFunctionType.Ln)
        f = work.tile([D, 1], F32)
        nc.scalar.activation(f, lnm, mybir.ActivationFunctionType.Exp, scale=-0.5)
        for kp in range(4):
            nc.vector.tensor_scalar_mul(
                k_sb[:, kp * 1024:(kp + 1) * 1024],
                k_sb[:, kp * 1024:(kp + 1) * 1024], f)

        # ---- flash attention, software-pipelined over (ic, jb) -----------
        # Emit S_T(n+1) before exp(n)/out(n) so the in-order PE queue keeps
        # streaming sim blocks while ACT computes the previous exp.
        steps = [(ic, jb) for ic in range(NIC) for jb in range(NJ)]

        def emit_st(n):
            ic, jb = steps[n]
            s_ps = psS.tile([128, IC], F32, tag="s")
            kb = k_sb[:, jb * 128:(jb + 1) * 128]
            for h2 in range(IC // 512):
                nc.tensor.matmul(
                    s_ps[:, h2 * 512:(h2 + 1) * 512], _mm(kb),
                    _mm(q_sb[:, ic * IC + h2 * 512: ic * IC + (h2 + 1) * 512]),
                    start=True, stop=True)
            return s_ps

        pending = []

        def queue_epilogue(ic, acc):
            # Positions are u-major (pos = u*32 + c), so each 32-wide slice
            # of acc is one u with c varying.  Transpose [33,32] slices (incl.
            # the denominator row) onto partition c, normalize per (c, u) on
            # DVE, and write straight into R_sb[:, u, :] -- no DMA.
            # The o33 copy is emitted NOW (frees the acc slot); the 8 work
            # items are queued and interleaved one-per-step into the next
            # chunk's flash steps.
            o33 = opool.tile([D + 1, IC], F32)
            # split the copy so the first transposes start half a copy early
            nc.vector.tensor_copy(o33[:, 0:IC // 2], acc[:, 0:IC // 2])
            nc.vector.tensor_copy(o33[:, IC // 2:], acc[:, IC // 2:])

            def tblock(g4, o33=o33, ic=ic):
                tps = psT.tile([D, 8, D + 1], F32, name="tps", tag="yp")
                for w in range(8):
                    m = g4 * 8 + w
                    nc.tensor.transpose(
                        tps[:, w, :], o33[:, m * 32:(m + 1) * 32], ident)
                return tps

            def nblock(tps, g4, ic=ic):
                rcp = work.tile([D, 8, 1], F32, tag="rcol", bufs=4)
                nc.vector.reciprocal(rcp, tps[:, :, D:D + 1])
                u0 = ic * 32 + g4 * 8
                nc.vector.tensor_mul(
                    R_sb[:, u0:u0 + 8, :], tps[:, :, 0:D],
                    rcp.broadcast_to((D, 8, D)))

            state = {}

            def phase1(g4):
                state[g4] = tblock(g4)

            def phase2(g4):
                nblock(state.pop(g4), g4)

            def phase3(mc, ncq, ic=ic):
                # output projection for one 512-col y block once its R rows
                # are in place.  The last chunk's copies go on ACT, which is
                # idle after the exp train drains (DVE is the tail straggler).
                yp = psT.tile([128, 512], F32, name="yp", tag="yp")
                nc.tensor.matmul(
                    yp, wo_sb[:, mc * 128:(mc + 1) * 128],
                    R_sb[:, ncq * 16:(ncq + 1) * 16, :],
                    start=True, stop=True)
                y_sb = work.tile([128, 512], F32, tag="y", bufs=4)
                if ic == NIC - 1:
                    nc.scalar.copy(y_sb, yp)
                else:
                    nc.vector.tensor_copy(y_sb, yp)
                nc.sync.dma_start(
                    y_d[mc * 128:(mc + 1) * 128,
                        ncq * 512:(ncq + 1) * 512], y_sb)

            # interleave: transpose+normalize a 512-col half, then project it
            for half in range(2):
                for g4 in (2 * half, 2 * half + 1):
                    pending.append(lambda g4=g4: phase1(g4))
                    pending.append(lambda g4=g4: phase2(g4))
                for mc in range(C // 128):
                    pending.append(
                        lambda mc=mc, half=half: phase3(mc, 2 * ic + half))

        noexp = bool(int(os.environ.get("NOEXP", "0")))
        e_const = None
        if noexp:
            e_const = const.tile([128, IC], F32R)
            nc.vector.memset(e_const.bitcast(F32), 1.0)

        accs = {}
        s_cur = emit_st(0)
        for n, (ic, jb) in enumerate(steps):
            if jb == 0:
                accs[ic] = psA.tile([D + 1, IC], F32, name="acc", tag="acc")
            s_next = emit_st(n + 1) if n + 1 < len(steps) else None
            if noexp:
                e = e_const
            else:
                e = epool.tile([128, IC], F32R)
                nc.scalar.activation(e, s_cur, mybir.ActivationFunctionType.Exp,
                                     scale=10.0)
            if pending:
                pending.pop(0)()
            vtb = vt_sb[:, jb // 4, jb % 4, :]
            acc = accs[ic]
            for h2 in range(IC // 512):
                nc.tensor.matmul(
                    acc[:, h2 * 512:(h2 + 1) * 512], _mm(vtb),
                    _mm(e[:, h2 * 512:(h2 + 1) * 512]),
                    start=(jb == 0), stop=(jb == NJ - 1))
            s_cur = s_next
            if jb == NJ - 1:
                queue_epilogue(ic, accs.pop(ic))
        while pending:
            pending.pop(0)()



def _build_program(repeat=1):
    key = ("nc", repeat)
    if key in _CACHE:
        return _CACHE[key], _CACHE[("names", repeat)]
    nc = bacc.Bacc("TRN2", target_bir_lowering=False, debug=False,
                   enable_asserts=False, num_devices=8)
    x_d = nc.dram_tensor("x", (C, L), BF16, kind="ExternalInput").ap()
    wqk_d = nc.dram_tensor("wqk", (C, 2 * D), BF16, kind="ExternalInput").ap()
    wv_d = nc.dram_tensor("wv", (C, D), BF16, kind="ExternalInput").ap()
    wo_d = nc.dram_tensor("wo", (D, C), F32R, kind="ExternalInput").ap()
    y_d = nc.dram_tensor("y", (C, L), F32, kind="ExternalOutput").ap()
    bodies = int(os.environ.get("BODIES", "1"))
    with tile.TileContext(nc) as tc:
        if repeat == 1:
            _emit(tc, y_d, x_d, wqk_d, wv_d, wo_d)
        else:
            with tc.For_i(0, repeat, 1):
                for _ in range(bodies):
                    _emit(tc, y_d, x_d, wqk_d, wv_d, wo_d)
    nc.compile()
    names = dict(x=x_d.name, wqk=wqk_d.name, wv=wv_d.name, wo=wo_d.name,
                 y=y_d.name)
    _CACHE[key] = nc
    _CACHE[("names", repeat)] = names
    return nc, names


def _in_maps(x, w_qkv, w_out, names):
    # x and the qkv weights ship as bf16 (fp8 fails tolerance: the 256-term
    # random-sign channel contraction keeps per-term quantization noise)
    bf16 = mybir.dt.np(BF16)
    maps = []
    for core in range(8):
        b, h = divmod(core, H)
        wq = w_qkv[h * D:(h + 1) * D]
        wk = w_qkv[128 + h * D:128 + (h + 1) * D]
        wv = w_qkv[256 + h * D:256 + (h + 1) * D]
        maps.append({
            names["x"]: np.ascontiguousarray(x[b]).astype(bf16),
            names["wqk"]: np.ascontiguousarray(
                np.concatenate([wq, wk], 0).T).astype(bf16),
            names["wv"]: np.ascontiguousarray(wv.T).astype(bf16),
            names["wo"]: np.ascontiguousarray(w_out[:, h * D:(h + 1) * D].T),
        })
    return maps


def run(x, w_qkv, w_out, b_out, **spmd_kwargs):
    """Build+run; returns (y_full, BassKernelResults)."""
    x = np.asarray(x, np.float32)
    w_qkv = np.asarray(w_qkv, np.float32)
    w_out = np.asarray(w_out, np.float32)
    b_out = np.asarray(b_out, np.float32)
    repeat = spmd_kwargs.pop("repeat", 1)
    nc, names = _build_program(repeat)
    res = bass_utils.run_bass_kernel_spmd(
        nc, _in_maps(x, w_qkv, w_out, names), core_ids=list(range(8)),
        **spmd_kwargs)
    y = np.zeros((B, C, L), np.float32)
    for core in range(8):
        y[core // H] += res.results[core][names["y"]]
    y += b_out[None, :, None]
    return y, res


def kernel(x, w_qkv, w_out, b_out):
    y, _ = run(x, w_qkv, w_out, b_out)
    return y



# revision 39
# speedup vs baseline: 1.2947x; 1.2947x over previous
"""Trainium2 Bass kernel for nn_Attention_18399639896530.

Reference computation (b=2, c=256, l=4096, heads=4, dim_head=32):
  qkv   = w_qkv @ x[b]                  (pointwise conv == channel matmul)
  q,k,v -> (b, h, d, l);  q,k L2-normalized over the *sequence* axis l
  sim   = 10 * q^T k    (per b,h: (l, l))
  attn  = softmax(sim, -1);  out = attn @ v^T   -> (b, h, l, d)
  y     = w_out @ out.reshape(b, 128, l) + b_out
          ^^^ row-major reshape of (h, l, d) -- a scrambled view, NOT a
          transpose: view[h*32+r', t] = out[b, h, r'*128 + t//32, t%32]

Sharding: 8 cores == 8 (b, h) pairs; per-core flash-style attention with the
softmax denominator produced by an extra ones-column in the stationary v^T
operand.  Both L2 norms fold into a single per-row scale of k (they both
scale the d-rows).

The scrambled output projection contracts over r' = i//128.  Query columns
are processed u-major (flash position u*32 + c <-> sequence index c*128 + u,
via a strided view of x in the q projection), so each 32-wide accumulator
slice holds one u with c = 0..31; small [33,32] TensorE transposes then land
o_norm/denominator directly on partition c, DVE normalizes into
R[r', u, dd] = o_norm[dd, r'*128+u], and y_h = wo_h^T.T @ R.  (A previous
revision instead DMA'd [128,32] tiles into single partitions of R -- that
partition-collapsing SBUF->SBUF DMA was 404us of a 486us kernel.)

x/w_qkv/w_v ship as bf16 (same 1 cyc/col as f32r, half the DMA; fp8 fails
the 2e-2 gate because the random-sign 256-channel contraction keeps the full
per-term quantization noise).  Projection square-norms accumulate on ACT
under the projection matmuls; ACT's spline table is warmed so the exp train
never reloads.  Host sums the 4 per-head partials per batch and adds b_out.
"""

import os
import sys
import numpy as np

try:
    import concourse  # noqa: F401
except ImportError:  # pragma: no cover
    sys.path.insert(0, "/opt/trn_rl_repo")

import concourse.bass as bass  # noqa: E402
import concourse.tile as tile  # noqa: E402
from concourse import bacc, mybir  # noqa: E402
from concourse import bass_utils  # noqa: E402
from concourse.masks import make_identity  # noqa: E402

B, C, L = 2, 256, 4096
H, D = 4, 32
IC = 1024          # i-chunk (query columns per block of the flash loop)
NIC = L // IC      # 4
NJ = L // 128      # 32 key blocks
F32 = mybir.dt.float32
F32R = mybir.dt.float32r   # single-pass fp32 matmul: 1 cyc/col at N>=256
BF16 = mybir.dt.bfloat16   # projection inputs ship as bf16: same 1 cyc/col
                           # as f32r, half the DMA + SBUF; fp8 was tried and
                           # fails tolerance (random-sign channel sum keeps
                           # the full ~4% per-term quantization noise)

_CACHE = {}
MM_F32 = bool(int(os.environ.get("MM_F32", "0")))


def _mm(ap):
    # hot-matmul operand dtype: f32r (1 cyc/col if real) vs plain f32 (4 cyc)
    return ap.bitcast(F32) if (MM_F32 and ap.dtype == F32R) else ap


def _emit(tc, y_d, x_d, wqk_d, wv_d, wo_d):
    from contextlib import ExitStack

    nc = tc.nc
    with ExitStack() as ctx:
        const = ctx.enter_context(tc.tile_pool(name="const", bufs=1))
        work = ctx.enter_context(tc.tile_pool(name="work", bufs=2))
        epool = ctx.enter_context(tc.tile_pool(name="epool", bufs=3))
        opool = ctx.enter_context(tc.tile_pool(name="opool", bufs=2))
        psA = ctx.enter_context(tc.tile_pool(name="psA", bufs=1, space="PSUM"))
        psS = ctx.enter_context(tc.tile_pool(name="psS", bufs=2, space="PSUM"))
        psT = ctx.enter_context(tc.tile_pool(name="psT", bufs=2, space="PSUM"))

        # ---- load inputs (small weights first so projection starts early)
        wqk_sb = const.tile([128, 2, 2 * D], BF16)
        nc.sync.dma_start(wqk_sb, wqk_d.rearrange("(cc p) o -> p cc o", p=128))
        wv_sb = const.tile([128, 2, D], BF16)
        nc.sync.dma_start(wv_sb, wv_d.rearrange("(cc p) o -> p cc o", p=128))
        wo_sb = const.tile([D, C], F32R)              # [r', o]
        nc.sync.dma_start(wo_sb, wo_d)
        x_sb = const.tile([128, 2, L], BF16)            # [c%128, c//128, l]
        xr = x_d.rearrange("(cc p) l -> p cc l", p=128)
        for lq in range(8):
            nc.sync.dma_start(x_sb[:, :, lq * 512:(lq + 1) * 512],
                              xr[:, :, lq * 512:(lq + 1) * 512])

        ones_f32 = const.tile([128, D], F32)
        nc.vector.memset(ones_f32, 1.0)
        # Ln-only table warm while the x DMA is in flight: narrows ACT's
        # possible table sets to the Ln-bearing ones, so the Square accums
        # and the f-chain Ln never reload; only the f-chain Exp pays one
        # ACT_TABLE_LOAD before the exp train.
        actwarm = work.tile([D, 1], F32, tag="actwarm")
        nc.scalar.activation(actwarm, ones_f32[0:D, 0:1],
                             mybir.ActivationFunctionType.Ln)
        warm_ps = psT.tile([D, D], F32, name="warm", tag="yp")
        for _ in range(40):
            nc.tensor.matmul(warm_ps, ones_f32, ones_f32, start=True, stop=True)
        ident = const.tile([D + 1, D + 1], F32)
        make_identity(nc, ident)

        # v^T blocks with a trailing ones column: [j%128, jb//4, jb%4, d(+1)]
        vt_sb = const.tile([128, NJ // 4, 4, D + 1], F32R)
        nc.vector.tensor_copy(
            vt_sb[:, :, :, D],
            ones_f32.rearrange("p (g l) -> p g l", l=4))

        q_sb = const.tile([D, L], F32R)
        k_sb = const.tile([D, L], F32R)
        # R[r', u, dd] = o_norm[dd, r'*128 + u]
        R_sb = const.tile([D, 128, D], F32R)

        # q columns are permuted u-major: flash position pos = u*32 + c maps
        # to sequence index i = c*128 + u.  A 32-wide slice of the attention
        # accumulator then holds one u with c=0..31, so a [33,32] transpose
        # lands o_norm directly on partition c = i//128 -- the layout the
        # output projection contracts over -- with no partition-collapsing
        # DMA.  x viewed with free dims (u, c): l = c*128 + u.
        x_perm = x_sb.rearrange("p cc (c u) -> p cc u c", c=D)

        # ---- q/k projection (k first: its chunks consume x in DMA order,
        # overlapping the x load; q's permuted view needs all of x).
        # Square-norm partials accumulate on ACT per chunk, hidden under the
        # projection matmuls instead of serializing before the flash loop.
        nq16 = work.tile([D, L // 512], F32, tag="nq16")
        nk16 = work.tile([D, L // 512], F32, tag="nk16")

        def proj_qk(which, dst, npart):
            for lc in range(L // 512):
                # alternate PSUM pools: 4 pq tiles in flight hides the
                # psum-slot -> copy -> free round-trip
                pool, tag = (psS, "s") if lc % 2 == 0 else (psT, "yp")
                pq = pool.tile([D, 512], F32, tag=tag, name="pq")
                for cc in range(2):
                    rhs = (x_perm[:, cc, lc * 16:(lc + 1) * 16, :]
                           if which == 0 else
                           x_sb[:, cc, lc * 512:(lc + 1) * 512])
                    nc.tensor.matmul(
                        pq, wqk_sb[:, cc, which * D:(which + 1) * D],
                        rhs, start=(cc == 0), stop=(cc == 1))
                sq_scr = work.tile([D, 512], F32, tag="sq")
                nc.scalar.activation(sq_scr, pq,
                                     mybir.ActivationFunctionType.Square,
                                     accum_out=npart[:, lc:lc + 1])
                nc.vector.tensor_copy(dst[:, lc * 512:(lc + 1) * 512], pq)

        # k and v interleaved per 512-col x chunk, so both consume x in DMA
        # order and hide entirely under the load; q last (its u-major
        # permuted view needs all of x).
        def proj_k_chunk(lc):
            pool, tag = (psS, "s") if lc % 2 == 0 else (psT, "yp")
            pq = pool.tile([D, 512], F32, tag=tag, name="pq")
            for cc in range(2):
                nc.tensor.matmul(
                    pq, wqk_sb[:, cc, D:2 * D],
                    x_sb[:, cc, lc * 512:(lc + 1) * 512],
                    start=(cc == 0), stop=(cc == 1))
            sq_scr = work.tile([D, 512], F32, tag="sq")
            nc.scalar.activation(sq_scr, pq,
                                 mybir.ActivationFunctionType.Square,
                                 accum_out=nk16[:, lc:lc + 1])
            nc.vector.tensor_copy(k_sb[:, lc * 512:(lc + 1) * 512], pq)

        def proj_v_group(g):
            vt_ps = psS.tile([128, 4, D], F32, tag="s")
            for l4 in range(4):
                jb = g * 4 + l4
                for cc in range(2):
                    nc.tensor.matmul(
                        vt_ps[:, l4, :],
                        x_sb[:, cc, jb * 128:(jb + 1) * 128], wv_sb[:, cc, :],
                        start=(cc == 0), stop=(cc == 1))
            nc.vector.tensor_copy(vt_sb[:, g, :, 0:D], vt_ps)

        for lc in range(8):
            proj_k_chunk(lc)
            proj_v_group(lc)
        proj_qk(0, q_sb, nq16)

        # ---- fold both L2 norms into one per-row scale of k --------------
        # f[d] = 1 / (max(||q_d||,eps) * max(||k_d||,eps))
        #      = exp(-0.5 * ln(max(sum q_d^2,eps^2) * max(sum k_d^2,eps^2)))
        nqs = work.tile([D, 1], F32)
        nks = work.tile([D, 1], F32)
        nc.vector.tensor_reduce(nqs, nq16, axis=mybir.AxisListType.X,
                                op=mybir.AluOpType.add)
        nc.vector.tensor_reduce(nks, nk16, axis=mybir.AxisListType.X,
                                op=mybir.AluOpType.add)
        nc.vector.tensor_scalar_max(nqs, nqs, 1e-24)
        nc.vector.tensor_scalar_max(nks, nks, 1e-24)
        m = work.tile([D, 1], F32)
        nc.vector.tensor_mul(m, nqs, nks)
        lnm = work.tile([D, 1], F32)
        nc.scalar.activation(lnm, m, mybir.ActivationFunctionType.Ln)
        f = work.tile([D, 1], F32)
        nc.scalar.activation(f, lnm, mybir.ActivationFunctionType.Exp, scale=-0.5)
        for kp in range(4):
            nc.vector.tensor_scalar_mul(
                k_sb[:, kp * 1024:(kp + 1) * 1024],
                k_sb[:, kp * 1024:(kp + 1) * 1024], f)

        # ---- flash attention, software-pipelined over (ic, jb) -----------
        # Emit S_T(n+1) before exp(n)/out(n) so the in-order PE queue keeps
        # streaming sim blocks while ACT computes the previous exp.
        steps = [(ic, jb) for ic in range(NIC) for jb in range(NJ)]

        def emit_st(n):
            ic, jb = steps[n]
            s_ps = psS.tile([128, IC], F32, tag="s")
            kb = k_sb[:, jb * 128:(jb + 1) * 128]
            for h2 in range(IC // 512):
                nc.tensor.matmul(
                    s_ps[:, h2 * 512:(h2 + 1) * 512], _mm(kb),
                    _mm(q_sb[:, ic * IC + h2 * 512: ic * IC + (h2 + 1) * 512]),
                    start=True, stop=True)
            return s_ps

        pending = []

        def queue_epilogue(ic, acc):
            # Positions are u-major (pos = u*32 + c), so each 32-wide slice
            # of acc is one u with c varying.  Transpose [33,32] slices (incl.
            # the denominator row) onto partition c, normalize per (c, u) on
            # DVE, and write straight into R_sb[:, u, :] -- no DMA.
            # The o33 copy is emitted NOW (frees the acc slot); the 8 work
            # items are queued and interleaved one-per-step into the next
            # chunk's flash steps.
            o33 = opool.tile([D + 1, IC], F32)
            # split the copy so the first transposes start half a copy early
            nc.vector.tensor_copy(o33[:, 0:IC // 2], acc[:, 0:IC // 2])
            nc.vector.tensor_copy(o33[:, IC // 2:], acc[:, IC // 2:])

            def tblock(g4, o33=o33, ic=ic):
                tps = psT.tile([D, 8, D + 1], F32, name="tps", tag="yp")
                for w in range(8):
                    m = g4 * 8 + w
                    nc.tensor.transpose(
                        tps[:, w, :], o33[:, m * 32:(m + 1) * 32], ident)
                return tps

            def nblock(tps, g4, ic=ic):
                rcp = work.tile([D, 8, 1], F32, tag="rcol", bufs=4)
                nc.vector.reciprocal(rcp, tps[:, :, D:D + 1])
                u0 = ic * 32 + g4 * 8
                nc.vector.tensor_mul(
                    R_sb[:, u0:u0 + 8, :], tps[:, :, 0:D],
                    rcp.broadcast_to((D, 8, D)))

            state = {}

            def phase1(g4):
                state[g4] = tblock(g4)

            def phase2(g4):
                nblock(state.pop(g4), g4)

            def phase3(mc, ncq, ic=ic):
                # output projection for one 512-col y block once its R rows
                # are in place.  The last chunk's copies go on ACT, which is
                # idle after the exp train drains (DVE is the tail straggler).
                yp = psT.tile([128, 512], F32, name="yp", tag="yp")
                nc.tensor.matmul(
                    yp, wo_sb[:, mc * 128:(mc + 1) * 128],
                    R_sb[:, ncq * 16:(ncq + 1) * 16, :],
                    start=True, stop=True)
                y_sb = work.tile([128, 512], F32, tag="y", bufs=4)
                if ic == NIC - 1:
                    nc.scalar.copy(y_sb, yp)
                else:
                    nc.vector.tensor_copy(y_sb, yp)
                nc.sync.dma_start(
                    y_d[mc * 128:(mc + 1) * 128,
                        ncq * 512:(ncq + 1) * 512], y_sb)

            # interleave: transpose+normalize a 512-col half, then project it
            for half in range(2):
                for g4 in (2 * half, 2 * half + 1):
                    pending.append(lambda g4=g4: phase1(g4))
                    pending.append(lambda g4=g4: phase2(g4))
                for mc in range(C // 128):
                    pending.append(
                        lambda mc=mc, half=half: phase3(mc, 2 * ic + half))

        noexp = bool(int(os.environ.get("NOEXP", "0")))
        e_const = None
        if noexp:
            e_const = const.tile([128, IC], F32R)
            nc.vector.memset(e_const.bitcast(F32), 1.0)

        accs = {}
        s_cur = emit_st(0)
        for n, (ic, jb) in enumerate(steps):
            if jb == 0:
                accs[ic] = psA.tile([D + 1, IC], F32, name="acc", tag="acc")
            s_next = emit_st(n + 1) if n + 1 < len(steps) else None
            if noexp:
                e = e_const
            else:
                e = epool.tile([128, IC], F32R)
                # split the exp into 512-col halves: doubles the in-flight
                # ACT instruction depth (HW exp latency is ~2x its occupancy)
                # and lets each av matmul start as soon as its half is ready
                for eh in range(IC // 512):
                    nc.scalar.activation(
                        e[:, eh * 512:(eh + 1) * 512],
                        s_cur[:, eh * 512:(eh + 1) * 512],
                        mybir.ActivationFunctionType.Exp, scale=10.0)
            if pending:
                pending.pop(0)()
            vtb = vt_sb[:, jb // 4, jb % 4, :]
            acc = accs[ic]
            for h2 in range(IC // 512):
                nc.tensor.matmul(
                    acc[:, h2 * 512:(h2 + 1) * 512], _mm(vtb),
                    _mm(e[:, h2 * 512:(h2 + 1) * 512]),
                    start=(jb == 0), stop=(jb == NJ - 1))
            s_cur = s_next
            if jb == NJ - 1:
                queue_epilogue(ic, accs.pop(ic))
        while pending:
            pending.pop(0)()



def _build_program(repeat=1):
    key = ("nc", repeat)
    if key in _CACHE:
        return _CACHE[key], _CACHE[("names", repeat)]
    nc = bacc.Bacc("TRN2", target_bir_lowering=False, debug=False,
                   enable_asserts=False, num_devices=8)
    x_d = nc.dram_tensor("x", (C, L), BF16, kind="ExternalInput").ap()
    wqk_d = nc.dram_tensor("wqk", (C, 2 * D), BF16, kind="ExternalInput").ap()
    wv_d = nc.dram_tensor("wv", (C, D), BF16, kind="ExternalInput").ap()
    wo_d = nc.dram_tensor("wo", (D, C), F32R, kind="ExternalInput").ap()
    y_d = nc.dram_tensor("y", (C, L), F32, kind="ExternalOutput").ap()
    bodies = int(os.environ.get("BODIES", "1"))
    with tile.TileContext(nc) as tc:
        if repeat == 1:
            _emit(tc, y_d, x_d, wqk_d, wv_d, wo_d)
        else:
            with tc.For_i(0, repeat, 1):
                for _ in range(bodies):
                    _emit(tc, y_d, x_d, wqk_d, wv_d, wo_d)
    nc.compile()
    names = dict(x=x_d.name, wqk=wqk_d.name, wv=wv_d.name, wo=wo_d.name,
                 y=y_d.name)
    _CACHE[key] = nc
    _CACHE[("names", repeat)] = names
    return nc, names


def _in_maps(x, w_qkv, w_out, names):
    # x and the qkv weights ship as bf16 (fp8 fails tolerance: the 256-term
    # random-sign channel contraction keeps per-term quantization noise)
    bf16 = mybir.dt.np(BF16)
    maps = []
    for core in range(8):
        b, h = divmod(core, H)
        wq = w_qkv[h * D:(h + 1) * D]
        wk = w_qkv[128 + h * D:128 + (h + 1) * D]
        wv = w_qkv[256 + h * D:256 + (h + 1) * D]
        maps.append({
            names["x"]: np.ascontiguousarray(x[b]).astype(bf16),
            names["wqk"]: np.ascontiguousarray(
                np.concatenate([wq, wk], 0).T).astype(bf16),
            names["wv"]: np.ascontiguousarray(wv.T).astype(bf16),
            names["wo"]: np.ascontiguousarray(w_out[:, h * D:(h + 1) * D].T),
        })
    return maps


def run(x, w_qkv, w_out, b_out, **spmd_kwargs):
    """Build+run; returns (y_full, BassKernelResults)."""
    x = np.asarray(x, np.float32)
    w_qkv = np.asarray(w_qkv, np.float32)
    w_out = np.asarray(w_out, np.float32)
    b_out = np.asarray(b_out, np.float32)
    repeat = spmd_kwargs.pop("repeat", 1)
    nc, names = _build_program(repeat)
    res = bass_utils.run_bass_kernel_spmd(
        nc, _in_maps(x, w_qkv, w_out, names), core_ids=list(range(8)),
        **spmd_kwargs)
    y = np.zeros((B, C, L), np.float32)
    for core in range(8):
        y[core // H] += res.results[core][names["y"]]
    y += b_out[None, :, None]
    return y, res


def kernel(x, w_qkv, w_out, b_out):
    y, _ = run(x, w_qkv, w_out, b_out)
    return y

